# revision 1
# baseline (speedup 1.0000x reference)
"""APPNP propagation (10 steps) on 8 TRN2 NeuronCores.

out = w0*feat + sum_{k=1..10} w_k * h_k,   h_k = Dd^-1/2 A Ds^-1/2 h_{k-1}

Distribution: destination nodes sharded 8 ways (12544/core); the propagated
state (pre-scaled g = h * src_norm) lives in a full f32 table replicated per
core, values bf16-rounded so a stride-2 bf16 bitcast view of gathered rows is
exact. Each step per core:
  - dma_gather (4 SWDGE queues) of the step's source rows; edge slots sorted
    by (subphase, src-stripe, dst-window), cells padded to a structure common
    to all 8 cores (SPMD: one program)
  - PE matmuls: one-hot fp8 scatter matrices S[slot, dst-rel] x bf16 view of
    gathered rows, accumulated in per-window PSUM slots across the 4 stripes
  - DVE eviction: next-table rows (x src_norm*dst_norm -> bf16) and output
    accumulation (+= w_k*dst_norm x R, f32)
  - SWDGE cast-pack bf16->f32 and an 8-core AllGather rebuilds the table.

Normalization is exact: norms fold into per-node scale columns applied at
eviction; S entries are exactly 1.0 in fp8; accumulation is f32 in PSUM. Only
one bf16 rounding of the state per step.
"""
import math
import os
import sys
import types
import numpy as np
import ml_dtypes

K_STEPS = int(os.environ.get("KM_STEPS", "10"))
BETA = 2.0
D = 64
NC = 8
WIN = 128                 # dst window width (= S columns, PSUM out partitions)
SHARD_WINDOWS = 98        # windows per core
SHARD = SHARD_WINDOWS * WIN   # 12544 dst rows per core
NTAB = NC * SHARD         # 100352 table rows
NSTRIPE = 4               # gather classes: (pair-stripe, src parity)
PAIRS = NTAB // 2         # bf16 table rows are node PAIRS of 128 values
PSTRIPE = PAIRS // 2      # 25088 (< 32768: int16-indexable)
SUBPHASES = tuple([8] * 12 + [2])   # windows per subphase (1 PSUM bank per window)
AG_SPLIT_SP = 12           # issue first AllGather after this subphase
AG_SPLIT_WIN = 98          # windows [0,98) in the first AllGather
SPLIT_ROWS = AG_SPLIT_WIN * WIN            # 10240 rows per core in group A
GROUP_A = NC * SPLIT_ROWS                  # table rows in group A


def _table_pos(node):
    """Node id -> table row, grouped so each split AllGather output is
    contiguous: group A = all ranks' rows [0, SPLIT_ROWS), then the tails."""
    node = np.asarray(node)
    c = node // SHARD
    r = node % SHARD
    return np.where(
        r < SPLIT_ROWS,
        c * SPLIT_ROWS + r,
        GROUP_A + c * (SHARD - SPLIT_ROWS) + (r - SPLIT_ROWS))
CALL = 1024               # gather idxs per dma_gather call (single_packet cap)

_LAST_EXEC_NS = None


def _install_prof_shim():
    """Provide antenv.axon_hooks so run_bass_kernel_spmd(trace=True) works."""
    if "antenv.axon_hooks" in sys.modules:
        return
    state = {"hook": None}
    mod = types.ModuleType("antenv.axon_hooks")
    mod.set_axon_ntff_profile_hook = lambda h: state.__setitem__("hook", h)
    mod.get_axon_ntff_profile_hook = lambda: state["hook"]
    sys.modules["antenv.axon_hooks"] = mod
    try:
        import antenv
        antenv.axon_hooks = mod
    except ImportError:
        pass
    try:
        from trn_agent_boot.trn_boot import _ntff_profile_via_ctypes
        hook = _ntff_profile_via_ctypes("/opt/axon/libaxon_pjrt.so")
        if hook is not None:
            mod.set_axon_ntff_profile_hook(hook)
    except Exception:
        pass
    from concourse import bass_utils
    bass_utils.upload_artifacts = lambda tmpdir: tmpdir


def _host_prep(feat, src, dst):
    """Index preprocessing: edge sharding/sorting, common loop structure,
    gather index tables, fp8 scatter matrices, scale columns."""
    n = feat.shape[0]
    src = np.asarray(src, dtype=np.int64)
    dst = np.asarray(dst, dtype=np.int64)
    feat = np.asarray(feat, dtype=np.float32)

    deg_out = np.bincount(src, minlength=NTAB).astype(np.float64)
    deg_in = np.bincount(dst, minlength=NTAB).astype(np.float64)
    src_norm = np.maximum(deg_out, 1.0) ** -0.5
    dst_norm = np.maximum(deg_in, 1.0) ** -0.5

    logs = [math.log(BETA + i) for i in range(1, K_STEPS + 2)]
    denom = sum(logs)
    w = [l / denom for l in logs]

    # table row i holds g = h * src_norm (bf16-rounded, f32 container)
    g0f = np.zeros((NTAB, D), dtype=np.float32)
    g0f[:n] = feat * src_norm[:n, None].astype(np.float32)
    g0n = g0f.astype(ml_dtypes.bfloat16)         # node-major, bf16-rounded
    g0f = g0n.astype(np.float32)                 # exact f32 copy for init
    g0 = np.zeros_like(g0n)                      # table-positioned bf16
    g0[_table_pos(np.arange(NTAB))] = g0n

    def col_layout(vec_core):  # [SHARD] -> [128, 98]; [p, w] = vec[w*128+p]
        return np.ascontiguousarray(
            vec_core.reshape(SHARD_WINDOWS, WIN).T.astype(np.float32))

    q = (src_norm * dst_norm).astype(np.float32)
    w0_inv = (w[0] / np.maximum(src_norm, 1e-30)).astype(np.float32)

    # per-core edges sorted by (window, stripe, dst)
    owner = dst // SHARD
    per_core = []
    for c in range(NC):
        m = owner == c
        s_c, d_c = src[m], dst[m]
        lw = (d_c - c * SHARD) // WIN
        tp = _table_pos(s_c)
        st = (tp // (2 * PSTRIPE)) * 2 + (tp & 1)
        order = np.lexsort((d_c, st, lw))
        s_c = tp  # downstream uses table positions
        per_core.append((s_c[order], d_c[order], lw[order], st[order]))

    sizes = np.zeros((NC, SHARD_WINDOWS, NSTRIPE), dtype=np.int64)
    for c in range(NC):
        _, _, lw, st = per_core[c]
        np.add.at(sizes[c], (lw, st), 1)
    chunks_ws = np.maximum(1, -(-sizes.max(axis=0) // 128))   # [98, 4]

    # processing blocks: (subphase, stripe)
    sp_bounds, start = [], 0
    for nwin in SUBPHASES:
        sp_bounds.append((start, start + nwin))
        start += nwin

    block_info, total_chunks = [], 0
    for sp_idx, (wa, wb) in enumerate(sp_bounds):
        for s in range(NSTRIPE):
            wins = list(range(wa, wb))
            nchunk = int(chunks_ws[wins, s].sum())
            nslot = nchunk * 128
            calls, off = [], 0
            while off < nslot:
                cn = min(CALL, nslot - off)
                calls.append((off, cn))
                off += cn
            block_info.append({
                "sp": sp_idx, "stripe": s, "wins": wins,
                "nchunk": nchunk, "nslot": nslot, "calls": calls,
                "chunk_off": total_chunks,
            })
            total_chunks += nchunk

    total_slots = total_chunks * 128
    fp8_one = np.float32(1.0).astype(ml_dtypes.float8_e4m3fn)
    idx_all = np.zeros((NC, total_slots), dtype=np.int16)
    smat_all = np.zeros((NC, total_chunks, 128, 128), dtype=ml_dtypes.float8_e4m3fn)

    for c in range(NC):
        s_c, d_c, lw_c, st_c = per_core[c]
        cnt = sizes[c]
        cell_start = np.zeros(SHARD_WINDOWS * NSTRIPE, dtype=np.int64)
        cell_start[1:] = np.cumsum(cnt.reshape(-1))[:-1]
        cell_start = cell_start.reshape(SHARD_WINDOWS, NSTRIPE)
        for bi in block_info:
            s = bi["stripe"]
            chunk_g = bi["chunk_off"]
            pos = bi["chunk_off"] * 128
            for wdx in bi["wins"]:
                ncell_chunks = int(chunks_ws[wdx, s])
                n_real = int(cnt[wdx, s])
                e0 = int(cell_start[wdx, s])
                cell_slots = ncell_chunks * 128
                loc = ((s_c[e0:e0 + n_real] // 2) % PSTRIPE).astype(np.int16)  # s_c = table pos
                idx_all[c, pos:pos + n_real] = loc
                rel = (d_c[e0:e0 + n_real] - c * SHARD - wdx * WIN).astype(np.int64)
                jj = np.arange(n_real)
                smat_all[c, chunk_g + jj // 128, jj % 128, rel] = fp8_one
                pos += cell_slots
                chunk_g += ncell_chunks

    # wrap idx stream per gather call: position i -> [i%16, i//16]; x8 groups
    idx_wrapped = np.zeros((NC, 128, total_slots // 16), dtype=np.int16)
    for bi in block_info:
        base = bi["chunk_off"] * 128
        for (off, cn) in bi["calls"]:
            a = base + off
            blk = idx_all[:, a:a + cn].reshape(NC, cn // 16, 16).transpose(0, 2, 1)
            idx_wrapped[:, :16, a // 16:(a + cn) // 16] = blk
    idx_wrapped[:, 16:, :] = np.tile(idx_wrapped[:, :16, :], (1, 7, 1))

    q_cols = np.stack([col_layout(q[c * SHARD:(c + 1) * SHARD]) for c in range(NC)])
    w0i_cols = np.stack([col_layout(w0_inv[c * SHARD:(c + 1) * SHARD]) for c in range(NC)])
    wdn_cols = np.zeros((NC, 128, K_STEPS * SHARD_WINDOWS), dtype=np.float32)
    for k in range(K_STEPS):
        wk = np.float32(w[k + 1])
        for c in range(NC):
            wdn_cols[c][:, k * SHARD_WINDOWS:(k + 1) * SHARD_WINDOWS] = \
                col_layout(dst_norm[c * SHARD:(c + 1) * SHARD].astype(np.float32) * wk)

    return {
        "g0": g0, "g0f": g0f, "idx": idx_wrapped, "smat": smat_all,
        "q_cols": q_cols, "w0i_cols": w0i_cols, "wdn_cols": wdn_cols,
        "blocks": block_info, "total_chunks": total_chunks,
        "chunks_ws": chunks_ws, "n": n,
    }


def _build_program(prep):
    from concourse import bacc, tile, mybir

    F32 = mybir.dt.float32
    BF16 = mybir.dt.bfloat16
    FP8 = mybir.dt.float8e4
    I16 = mybir.dt.int16

    blocks = prep["blocks"]
    total_chunks = prep["total_chunks"]
    chunks_ws = prep["chunks_ws"]
    total_slots = total_chunks * 128

    nc = bacc.Bacc(None, target_bir_lowering=False, num_swdge_queues=4)

    tab0 = nc.declare_dram_parameter("tab0", [PAIRS, 2 * D], BF16, isOutput=False)
    idx_in = nc.declare_dram_parameter("idx", [128, total_slots // 16], I16, isOutput=False)
    smat_in = nc.declare_dram_parameter("smat", [128, total_chunks, 128], FP8, isOutput=False)
    qv_in = nc.declare_dram_parameter("qv", [128, SHARD_WINDOWS], F32, isOutput=False)
    w0i_in = nc.declare_dram_parameter("w0i", [128, SHARD_WINDOWS], F32, isOutput=False)
    wdn_in = nc.declare_dram_parameter("wdn", [128, K_STEPS * SHARD_WINDOWS], F32, isOutput=False)
    g0sh_in = nc.declare_dram_parameter("g0sh", [128, SHARD_WINDOWS, D], F32, isOutput=False)
    out_ext = nc.declare_dram_parameter("out", [SHARD, D], F32, isOutput=True)

    tab_a = nc.dram_tensor("tab_a", [PAIRS, 2 * D], BF16)
    tabs = [tab_a, tab_a]
    ag_in = nc.dram_tensor("ag_in", [SHARD, D], BF16)

    with tile.TileContext(nc) as tc:
        with (
            tc.tile_pool(name="persist", bufs=1) as pp,
            tc.tile_pool(name="sstage", bufs=2) as s_pool,
            tc.tile_pool(name="gstage", bufs=4) as gp,
            tc.tile_pool(name="psum", bufs=1, space="PSUM") as psum_pool,
        ):
            idx_t = pp.tile([128, total_slots // 16], I16)
            nc.sync.dma_start(idx_t[:], idx_in[:])
            qv = pp.tile([128, SHARD_WINDOWS], F32)
            nc.sync.dma_start(qv[:], qv_in[:])
            w0i = pp.tile([128, SHARD_WINDOWS], F32)
            nc.sync.dma_start(w0i[:], w0i_in[:])
            wdn = pp.tile([128, K_STEPS * SHARD_WINDOWS], F32)
            nc.sync.dma_start(wdn[:], wdn_in[:])
            acc = pp.tile([128, SHARD_WINDOWS, D], F32)
            hnew = pp.tile([128, SHARD_WINDOWS, D], BF16)
            g0sh = pp.tile([128, SHARD_WINDOWS, D], F32)
            nc.sync.dma_start(g0sh[:], g0sh_in[:])

            nc.sync.dma_start(tabs[0][:], tab0[:])

            sp_first_win = []
            _w = 0
            for _nwin in SUBPHASES:
                sp_first_win.append(_w)
                _w += _nwin

            for wdx in range(SHARD_WINDOWS):
                nc.vector.tensor_scalar_mul(
                    acc[:, wdx, :], g0sh[:, wdx, :], w0i[:, wdx:wdx + 1])

            for k in range(K_STEPS):
                for sp_idx in range(len(SUBPHASES)):
                    nwin = SUBPHASES[sp_idx]
                    wbase = sp_first_win[sp_idx]
                    sp_blocks = [bi for bi in blocks if bi["sp"] == sp_idx]
                    sp_c0 = sp_blocks[0]["chunk_off"]
                    sp_nchunk = sum(bi["nchunk"] for bi in sp_blocks)
                    st_sp = s_pool.tile([128, sp_nchunk, 128], FP8, tag="ss",
                                        name=f"ss{k}_{sp_idx}")
                    nc.sync.dma_start(
                        st_sp[:], smat_in[:, sp_c0:sp_c0 + sp_nchunk, :])
                    gtiles_all, cell_off, blk_c0 = {}, {}, {}
                    for bi in sp_blocks:
                        s_ = bi["stripe"]
                        blk_c0[s_] = bi["chunk_off"] - sp_c0
                        base_slot = bi["chunk_off"] * 128
                        gts = []
                        for (off, cn) in bi["calls"]:
                            g = gp.tile([128, CALL // 128, 2 * D], BF16, tag=f"g{s_}",
                                        name=f"g{k}_{sp_idx}_{s_}_{len(gts)}")
                            a = base_slot + off
                            nc.gpsimd.dma_gather(
                                g[:, :cn // 128, :],
                                tabs[k % 2][(s_ // 2) * PSTRIPE:(s_ // 2 + 1) * PSTRIPE, :],
                                idx_t[:, a // 16:(a + cn) // 16],
                                num_idxs=cn, num_idxs_reg=cn, elem_size=2 * D,
                                single_packet=True,
                                queue_num=s_,
                            )
                            gts.append(g)
                        gtiles_all[s_] = gts
                        # chunk offset of each window's cell within the block
                        co, cur = {}, 0
                        for wdx in bi["wins"]:
                            co[wdx] = cur
                            cur += int(chunks_ws[wdx, s_])
                        cell_off[s_] = co
                    # window-major matmuls: contiguous accumulation group
                    for li in range(nwin):
                        wdx = wbase + li
                        bank = psum_pool.tile([128, 512], F32, tag=f"pb{li}",
                                              name=f"pb{k}_{sp_idx}_{li}")
                        first = True
                        for s_ in range(NSTRIPE):
                            ncc = int(chunks_ws[wdx, s_])
                            coff = cell_off[s_][wdx]
                            par = s_ % 2
                            for j in range(ncc):
                                call_i, col = divmod(coff + j, CALL // 128)
                                g = gtiles_all[s_][call_i]
                                rhs = g[:, col, par * D:(par + 1) * D]
                                is_last = (s_ == NSTRIPE - 1) and (j == ncc - 1)
                                nc.tensor.matmul(
                                    bank[:, 0:64],
                                    st_sp[:, blk_c0[s_] + coff + j, :],
                                    rhs,
                                    start=first,
                                    stop=is_last,
                                    skip_group_check=True,
                                )
                                first = False
                        nc.vector.tensor_scalar_mul(
                            hnew[:, wdx, :], bank[:, 0:64], qv[:, wdx:wdx + 1])
                        nc.vector.scalar_tensor_tensor(
                            acc[:, wdx, :], bank[:, 0:64],
                            wdn[:, k * SHARD_WINDOWS + wdx:k * SHARD_WINDOWS + wdx + 1],
                            acc[:, wdx, :],
                            op0=mybir.AluOpType.mult, op1=mybir.AluOpType.add)

                    if (k < K_STEPS - 1 and sp_idx in (AG_SPLIT_SP, len(SUBPHASES) - 1)
                            and not (sp_idx != AG_SPLIT_SP and AG_SPLIT_WIN >= SHARD_WINDOWS)):
                        if sp_idx == AG_SPLIT_SP:
                            w0_, w1_ = 0, AG_SPLIT_WIN
                        else:
                            w0_, w1_ = AG_SPLIT_WIN, SHARD_WINDOWS
                        rows = slice(w0_ * WIN, w1_ * WIN)
                        nc.sync.dma_start(
                            ag_in[rows, :].rearrange("(a p) d -> p a d", p=WIN),
                            hnew[:, w0_:w1_, :])
                        tab_nodes = tabs[(k + 1) % 2][:].rearrange(
                            "a (two d) -> (a two) d", two=2)
                        t0 = NC * w0_ * WIN
                        t1 = t0 + NC * (w1_ - w0_) * WIN
                        nc.gpsimd.collective_compute(
                            "AllGather", mybir.AluOpType.bypass,
                            replica_groups=[list(range(NC))],
                            ins=[ag_in[rows, :].opt()],
                            outs=[tab_nodes[t0:t1, :].opt()],
                        )

            nc.sync.dma_start(
                out_ext[:].rearrange("(a p) d -> p a d", p=WIN), acc[:])

    nc.compile()
    return nc


def kernel(feat, src, dst):
    global _LAST_EXEC_NS
    _install_prof_shim()
    from concourse import bass_utils

    feat = np.asarray(feat, dtype=np.float32)
    prep = _host_prep(feat, np.asarray(src), np.asarray(dst))
    nc = _build_program(prep)

    in_maps = []
    for c in range(NC):
        g0sh = prep["g0f"][c * SHARD:(c + 1) * SHARD].reshape(SHARD_WINDOWS, WIN, D)
        g0sh = np.ascontiguousarray(g0sh.transpose(1, 0, 2))
        in_maps.append({
            "tab0": prep["g0"].reshape(PAIRS, 2 * D),
            "idx": prep["idx"][c],
            "smat": np.ascontiguousarray(prep["smat"][c].transpose(1, 0, 2)),
            "qv": prep["q_cols"][c],
            "w0i": prep["w0i_cols"][c],
            "wdn": prep["wdn_cols"][c],
            "g0sh": g0sh,
        })

    res = bass_utils.run_bass_kernel_spmd(
        nc, in_maps, core_ids=list(range(NC)), trace=True)
    _LAST_EXEC_NS = res.exec_time_ns

    full = np.concatenate([res.results[c]["out"] for c in range(NC)], axis=0)
    return full[:prep["n"]].astype(np.float32)

